# revision 1
# baseline (speedup 1.0000x reference)
"""Item2Vec negative-sampling loss on 8 Trainium2 NeuronCores.

Strategy (data-parallel over batch, tables replicated per core):
  - Each core handles B/8 = 2048 batch elements (16 tiles of 128).
  - bf16 tables in HBM; indirect-DMA gathers sized on a ramp
    [1,1,2,4,4,2,1,1] tiles so the first multiply starts early and the
    last gather leaves only one tile of compute in the tail.
  - Per 128-batch tile:
      DVE:  prod = pn_embs * broadcast(center_emb), written PERMUTED so
            each of the 8 d-slices is a contiguous [128, 336] block
      PE :  8 accumulating identity-matmuls fold d by 8 into PSUM
            [128, 21*16] f32 (plus a warmup matmul burst at t=0 to get
            the tensor engine clock boosted before real work)
      DVE:  tensor_reduce psum [128,21,16] -> scores
  - Scalar engine: per-group sigmoids (scale +/-1, sigmoid table stays
    resident), then ONE final Ln(sig+eps) with accum_out -> colsum.
    (Interleaving Ln per group costs a 1.3us ACT table reload each way.)
  - colsum -> ones-matmul -> scalar partial; host sums 8 partials.
"""

import numpy as np
import ml_dtypes
from contextlib import ExitStack

import concourse.bass as bass
import concourse.bacc as bacc
import concourse.mybir as mybir
import concourse.tile as tile
from concourse.bass_utils import run_bass_kernel_spmd

V = 1_000_000
D = 128
B = 16384
NEG = 20
P = 128
NCORES = 8
BPC = B // NCORES           # 2048 batch elements per core
NTILES = BPC // P           # 16 batch tiles per core
K1 = NEG + 1                # pos + negs per batch element

GROUPS = [1, 1, 2, 4, 4, 2, 1, 1]   # tiles per pos/neg gather
GSTART = [sum(GROUPS[:i]) for i in range(len(GROUPS))]
MAXG = max(GROUPS)
C_GRP = 8                   # batch tiles per center gather instruction
N_C = NTILES // C_GRP       # 2 gather instructions from W_in

NSLC = 8                    # d-slices folded by PE identity-matmuls
SLC = D // NSLC             # 16
WARMUP_MM = 24              # PE warmup matmuls at t=0

DT_TAB = mybir.dt.bfloat16
NP_TAB = ml_dtypes.bfloat16

TRACE = False
LAST_RESULTS = None

_NC = None


def _body(ctx, tc, w_in, w_out, cidx, pnidx, ident, out):
    nc = tc.nc
    f32 = mybir.dt.float32
    X = mybir.AxisListType.X
    AF = mybir.ActivationFunctionType

    idx_pool = ctx.enter_context(tc.tile_pool(name="idx", bufs=1))
    pn_pool = ctx.enter_context(tc.tile_pool(name="pn", bufs=3))
    pr_pool = ctx.enter_context(tc.tile_pool(name="pr", bufs=3))
    c_pool = ctx.enter_context(tc.tile_pool(name="c", bufs=1))
    sc_pool = ctx.enter_context(tc.tile_pool(name="sc", bufs=1))
    ps_pool = ctx.enter_context(tc.tile_pool(name="ps", bufs=4, space="PSUM"))
    wm_pool = ctx.enter_context(tc.tile_pool(name="wm", bufs=1, space="PSUM"))

    # Offset tiles: contiguous per gather (HW SWDGE mishandles strided
    # offset APs). Loads split between Sync and Scalar HWDGE queues.
    idt = sc_pool.tile([P, P], DT_TAB, tag="idt")
    c_offs = [idx_pool.tile([P, C_GRP], mybir.dt.int32, tag=f"coff{g}",
                            name=f"coff{g}") for g in range(N_C)]
    pn_offs = [idx_pool.tile([P, GROUPS[g] * K1], mybir.dt.int32,
                             tag=f"pnoff{g}", name=f"pnoff{g}")
               for g in range(len(GROUPS))]

    def pn_idx_slice(g):
        a = GSTART[g] * K1
        return pnidx[:, a:a + GROUPS[g] * K1]

    nc.sync.dma_start(out=idt[:], in_=ident[:, :])
    nc.scalar.dma_start(out=pn_offs[0][:], in_=pn_idx_slice(0))
    nc.sync.dma_start(out=c_offs[0][:], in_=cidx[0, :, :])
    nc.scalar.dma_start(out=pn_offs[1][:], in_=pn_idx_slice(1))
    nc.sync.dma_start(out=pn_offs[2][:], in_=pn_idx_slice(2))
    nc.scalar.dma_start(out=pn_offs[3][:], in_=pn_idx_slice(3))
    nc.sync.dma_start(out=pn_offs[4][:], in_=pn_idx_slice(4))
    nc.scalar.dma_start(out=c_offs[1][:], in_=cidx[1, :, :])
    nc.sync.dma_start(out=pn_offs[5][:], in_=pn_idx_slice(5))
    nc.scalar.dma_start(out=pn_offs[6][:], in_=pn_idx_slice(6))
    nc.sync.dma_start(out=pn_offs[7][:], in_=pn_idx_slice(7))

    eps_t = sc_pool.tile([P, 1], f32, tag="eps")
    nc.any.memset(eps_t[:], 1e-10)
    ones = sc_pool.tile([P, 1], f32, tag="ones")
    nc.any.memset(ones[:], 1.0)

    # PE warmup burst: gets the tensor-engine clock boosted during the
    # gather ramp (cold-start matmuls run ~1.6x slower).
    warm = wm_pool.tile([P, P], f32, tag="warm")
    for i in range(WARMUP_MM):
        nc.tensor.matmul(out=warm[:], lhsT=idt[:], rhs=idt[:],
                         start=(i == 0), stop=(i == WARMUP_MM - 1))
    wscr = sc_pool.tile([1, 1], f32, tag="wscr")
    nc.vector.tensor_copy(out=wscr[:], in_=warm[0:1, 0:1])

    scores = sc_pool.tile([P, NTILES * K1], f32, tag="scores")
    sig = sc_pool.tile([P, NTILES * K1], f32, tag="sig")
    lg = sc_pool.tile([P, NTILES * K1], f32, tag="lg")

    c_tiles = [c_pool.tile([P, C_GRP * D], DT_TAB, tag=f"ctile{g}",
                           name=f"ctile{g}") for g in range(N_C)]

    def gather_c(g):
        nc.gpsimd.indirect_dma_start(
            out=c_tiles[g][:], out_offset=None, in_=w_in[:, :],
            in_offset=bass.IndirectOffsetOnAxis(ap=c_offs[g][:, :], axis=0))

    gather_c(0)

    for g, ng in enumerate(GROUPS):
        pnt = pn_pool.tile([P, MAXG * K1 * D], DT_TAB, tag="pnt")
        nc.gpsimd.indirect_dma_start(
            out=pnt[:, :ng * K1 * D], out_offset=None, in_=w_out[:, :],
            in_offset=bass.IndirectOffsetOnAxis(ap=pn_offs[g][:, :], axis=0))
        if GSTART[g] + ng == C_GRP:
            gather_c(1)
        for j in range(ng):
            t = GSTART[g] + j
            gi, lj = t // C_GRP, t % C_GRP
            ctv = c_tiles[gi][:, lj * D:(lj + 1) * D]
            # multiply, writing the product PERMUTED: linear layout
            # q*336 + k*16 + s so each d-slice q is contiguous [128,336]
            pv = pnt[:, j * K1 * D:(j + 1) * K1 * D].rearrange(
                "p (k q s) -> p k q s", k=K1, q=NSLC)
            prm = pr_pool.tile([P, K1 * D], DT_TAB, tag="prm")
            po = prm[:].rearrange("p (q k s) -> p k q s", q=NSLC, k=K1)
            cb = (ctv.rearrange("p (q s) -> p q s", q=NSLC)
                  .unsqueeze(1).broadcast_to([P, K1, NSLC, SLC]))
            nc.vector.tensor_tensor(out=po, in0=pv, in1=cb,
                                    op=mybir.AluOpType.mult)
            acc = ps_pool.tile([P, K1 * SLC], f32, tag="acc", name=f"acc{t}")
            for q in range(NSLC):
                nc.tensor.matmul(
                    out=acc[:], lhsT=idt[:],
                    rhs=prm[:, q * K1 * SLC:(q + 1) * K1 * SLC],
                    start=(q == 0), stop=(q == NSLC - 1))
            nc.vector.tensor_reduce(
                out=scores[:, t * K1:(t + 1) * K1],
                in_=acc[:].rearrange("p (k d) -> p k d", k=K1),
                axis=X, op=mybir.AluOpType.add)
        # this group's sigmoids (sigmoid table stays resident on ACT)
        w = ng * K1
        sl = slice(GSTART[g] * K1, GSTART[g] * K1 + w)
        s3 = scores[:, sl].rearrange("p (t k) -> p t k", k=K1)
        g3 = sig[:, sl].rearrange("p (t k) -> p t k", k=K1)
        nc.scalar.activation(out=g3[:, :, 0:1], in_=s3[:, :, 0:1],
                             func=AF.Sigmoid, scale=1.0)
        nc.scalar.activation(out=g3[:, :, 1:K1], in_=s3[:, :, 1:K1],
                             func=AF.Sigmoid, scale=-1.0)

    colsum = sc_pool.tile([P, 1], f32, tag="colsum")
    nc.scalar.activation(out=lg[:], in_=sig[:], func=AF.Ln, bias=eps_t[:],
                         accum_out=colsum[:])
    acc_ps = wm_pool.tile([1, 1], f32, tag="accf")
    nc.tensor.matmul(out=acc_ps[:], lhsT=colsum[:], rhs=ones[:],
                     start=True, stop=True)
    res = sc_pool.tile([1, 1], f32, tag="res")
    nc.vector.tensor_copy(out=res[:], in_=acc_ps[:])
    nc.sync.dma_start(out=out[:, :], in_=res[:])


def _build():
    nc = bacc.Bacc("TRN2", target_bir_lowering=False, debug=False)
    w_in = nc.dram_tensor("w_in", [V, D], DT_TAB, kind="ExternalInput")
    w_out = nc.dram_tensor("w_out", [V, D], DT_TAB, kind="ExternalInput")
    cidx = nc.dram_tensor("cidx", [N_C, P, C_GRP], mybir.dt.int32,
                          kind="ExternalInput")
    pnidx = nc.dram_tensor("pnidx", [P, NTILES * K1], mybir.dt.int32,
                           kind="ExternalInput")
    ident = nc.dram_tensor("ident", [P, P], DT_TAB, kind="ExternalInput")
    out = nc.dram_tensor("out", [1, 1], mybir.dt.float32,
                         kind="ExternalOutput")
    with tile.TileContext(nc) as tc:
        with ExitStack() as ctx:
            _body(ctx, tc, w_in, w_out, cidx, pnidx, ident, out)
    nc.compile()
    return nc


def _get_nc():
    global _NC
    if _NC is None:
        _NC = _build()
    return _NC


def _make_in_maps(inputs):
    center = np.asarray(inputs["center"]).astype(np.int32)
    pos = np.asarray(inputs["pos"]).astype(np.int32)
    neg = np.asarray(inputs["neg"]).astype(np.int32)
    wi = np.asarray(inputs["W_in"]).astype(NP_TAB)
    wo = np.asarray(inputs["W_out"]).astype(NP_TAB)
    ident = np.eye(P, dtype=NP_TAB)

    in_maps = []
    for c in range(NCORES):
        sl = slice(c * BPC, (c + 1) * BPC)
        # ce[g, p, j] = center index of batch tile t=g*C_GRP+j, partition p
        ce = center[sl].reshape(N_C, C_GRP, P).transpose(0, 2, 1)
        ce = np.ascontiguousarray(ce)
        # pn[p, t*K1+k] = pos/neg index of batch tile t, partition p
        pn = np.empty((NTILES, P, K1), np.int32)
        pn[:, :, 0] = pos[sl].reshape(NTILES, P)
        pn[:, :, 1:] = neg[sl].reshape(NTILES, P, NEG)
        pn = np.ascontiguousarray(
            pn.transpose(1, 0, 2).reshape(P, NTILES * K1))
        in_maps.append({"w_in": wi, "w_out": wo, "cidx": ce, "pnidx": pn,
                        "ident": ident})
    return in_maps


def kernel(center, pos, neg, W_in, W_out):
    global LAST_RESULTS
    in_maps = _make_in_maps(dict(center=center, pos=pos, neg=neg,
                                 W_in=W_in, W_out=W_out))
    nc = _get_nc()
    br = run_bass_kernel_spmd(nc, in_maps, core_ids=list(range(NCORES)),
                              trace=TRACE)
    LAST_RESULTS = br
    total = sum(float(r["out"][0, 0]) for r in br.results)
    return np.float32(-total / B)



# revision 9
# speedup vs baseline: 1.1715x; 1.1715x over previous
"""Item2Vec negative-sampling loss on 8 Trainium2 NeuronCores.

Strategy (data-parallel over batch, tables replicated per core):
  - Each core handles B/8 = 2048 batch elements (16 tiles of 128).
  - bf16 tables in HBM; indirect-DMA gathers sized on a ramp
    [1,1,2,4,4,2,1,1] tiles so the first multiply starts early and the
    last gather leaves only one tile of compute in the tail.
  - Per 128-batch tile:
      DVE:  prod = pn_embs * broadcast(center_emb), written PERMUTED so
            each of the 8 d-slices is a contiguous [128, 336] block
      PE :  8 accumulating identity-matmuls fold d by 8 into PSUM
            [128, 21*16] f32 (plus a warmup matmul burst at t=0 to get
            the tensor engine clock boosted before real work)
      DVE:  tensor_reduce psum [128,21,16] -> scores
  - Scalar engine: per-group sigmoids (scale +/-1, sigmoid table stays
    resident), then ONE final Ln(sig+eps) with accum_out -> colsum.
    (Interleaving Ln per group costs a 1.3us ACT table reload each way.)
  - colsum -> ones-matmul -> scalar partial; host sums 8 partials.
"""

import numpy as np
import ml_dtypes
from contextlib import ExitStack

import concourse.bass as bass
import concourse.bacc as bacc
import concourse.mybir as mybir
import concourse.tile as tile
from concourse.bass_utils import run_bass_kernel_spmd

V = 1_000_000
D = 128
B = 16384
NEG = 20
P = 128
NCORES = 8
BPC = B // NCORES           # 2048 batch elements per core
NTILES = BPC // P           # 16 batch tiles per core
K1 = NEG + 1                # pos + negs per batch element

GROUPS = [1, 1, 2, 4, 4, 2, 1, 1]   # tiles per pos/neg gather
GSTART = [sum(GROUPS[:i]) for i in range(len(GROUPS))]
MAXG = max(GROUPS)
C_GRP = 8                   # batch tiles per center gather instruction
N_C = NTILES // C_GRP       # 2 gather instructions from W_in

NSLC = 8                    # d-slices folded by PE identity-matmuls
SLC = D // NSLC             # 16
WARMUP_MM = 24              # PE warmup matmuls at t=0

DT_TAB = mybir.dt.bfloat16
NP_TAB = ml_dtypes.bfloat16

TRACE = False
LAST_RESULTS = None

_NC = None


def _body(ctx, tc, w_in, w_out, cidx, pnidx, ident, out):
    nc = tc.nc
    f32 = mybir.dt.float32
    X = mybir.AxisListType.X
    AF = mybir.ActivationFunctionType

    idx_pool = ctx.enter_context(tc.tile_pool(name="idx", bufs=1))
    pn_pool = ctx.enter_context(tc.tile_pool(name="pn", bufs=3))
    pr_pool = ctx.enter_context(tc.tile_pool(name="pr", bufs=3))
    c_pool = ctx.enter_context(tc.tile_pool(name="c", bufs=1))
    sc_pool = ctx.enter_context(tc.tile_pool(name="sc", bufs=1))
    ps_pool = ctx.enter_context(tc.tile_pool(name="ps", bufs=4, space="PSUM"))
    wm_pool = ctx.enter_context(tc.tile_pool(name="wm", bufs=1, space="PSUM"))

    # Offset tiles: contiguous per gather (HW SWDGE mishandles strided
    # offset APs). Loads split between Sync and Scalar HWDGE queues.
    idt = sc_pool.tile([P, P], DT_TAB, tag="idt")
    c_offs = [idx_pool.tile([P, C_GRP], mybir.dt.int32, tag=f"coff{g}",
                            name=f"coff{g}") for g in range(N_C)]
    pn_offs = [idx_pool.tile([P, GROUPS[g] * K1], mybir.dt.int32,
                             tag=f"pnoff{g}", name=f"pnoff{g}")
               for g in range(len(GROUPS))]

    def pn_idx_slice(g):
        a = GSTART[g] * K1
        return pnidx[:, a:a + GROUPS[g] * K1]

    nc.sync.dma_start(out=idt[:], in_=ident[:, :])
    nc.scalar.dma_start(out=pn_offs[0][:], in_=pn_idx_slice(0))
    nc.sync.dma_start(out=c_offs[0][:], in_=cidx[0, :, :])
    nc.scalar.dma_start(out=pn_offs[1][:], in_=pn_idx_slice(1))
    nc.sync.dma_start(out=pn_offs[2][:], in_=pn_idx_slice(2))
    nc.scalar.dma_start(out=pn_offs[3][:], in_=pn_idx_slice(3))
    nc.sync.dma_start(out=pn_offs[4][:], in_=pn_idx_slice(4))
    nc.scalar.dma_start(out=c_offs[1][:], in_=cidx[1, :, :])
    nc.sync.dma_start(out=pn_offs[5][:], in_=pn_idx_slice(5))
    nc.scalar.dma_start(out=pn_offs[6][:], in_=pn_idx_slice(6))
    nc.sync.dma_start(out=pn_offs[7][:], in_=pn_idx_slice(7))

    eps_t = sc_pool.tile([P, 1], f32, tag="eps")
    nc.any.memset(eps_t[:], 1e-10)
    ones = sc_pool.tile([P, 1], f32, tag="ones")
    nc.any.memset(ones[:], 1.0)

    # PE warmup burst: gets the tensor-engine clock boosted during the
    # gather ramp (cold-start matmuls run ~1.6x slower).
    warm = wm_pool.tile([P, P], f32, tag="warm")
    for i in range(WARMUP_MM):
        nc.tensor.matmul(out=warm[:], lhsT=idt[:], rhs=idt[:],
                         start=(i == 0), stop=(i == WARMUP_MM - 1))
    wscr = sc_pool.tile([1, 1], f32, tag="wscr")
    nc.vector.tensor_copy(out=wscr[:], in_=warm[0:1, 0:1])

    scores = sc_pool.tile([P, NTILES * K1], f32, tag="scores")
    sig = sc_pool.tile([P, NTILES * K1], f32, tag="sig")
    lg = sc_pool.tile([P, NTILES * K1], f32, tag="lg")

    c_tiles = [c_pool.tile([P, C_GRP * D], DT_TAB, tag=f"ctile{g}",
                           name=f"ctile{g}") for g in range(N_C)]

    def gather_c(g):
        nc.gpsimd.indirect_dma_start(
            out=c_tiles[g][:], out_offset=None, in_=w_in[:, :],
            in_offset=bass.IndirectOffsetOnAxis(ap=c_offs[g][:, :], axis=0))

    gather_c(0)

    for g, ng in enumerate(GROUPS):
        pnt = pn_pool.tile([P, MAXG * K1 * D], DT_TAB, tag="pnt")
        nc.gpsimd.indirect_dma_start(
            out=pnt[:, :ng * K1 * D], out_offset=None, in_=w_out[:, :],
            in_offset=bass.IndirectOffsetOnAxis(ap=pn_offs[g][:, :], axis=0))
        if GSTART[g] + ng == C_GRP:
            gather_c(1)
        for j in range(ng):
            t = GSTART[g] + j
            gi, lj = t // C_GRP, t % C_GRP
            ctv = c_tiles[gi][:, lj * D:(lj + 1) * D]
            # multiply, writing the product PERMUTED: linear layout
            # q*336 + k*16 + s so each d-slice q is contiguous [128,336]
            pv = pnt[:, j * K1 * D:(j + 1) * K1 * D].rearrange(
                "p (k q s) -> p k q s", k=K1, q=NSLC)
            prm = pr_pool.tile([P, K1 * D], DT_TAB, tag="prm")
            po = prm[:].rearrange("p (q k s) -> p k q s", q=NSLC, k=K1)
            cb = (ctv.rearrange("p (q s) -> p q s", q=NSLC)
                  .unsqueeze(1).broadcast_to([P, K1, NSLC, SLC]))
            nc.vector.tensor_tensor(out=po, in0=pv, in1=cb,
                                    op=mybir.AluOpType.mult)
            acc = ps_pool.tile([P, K1 * SLC], f32, tag="acc", name=f"acc{t}")
            for q in range(NSLC):
                nc.tensor.matmul(
                    out=acc[:], lhsT=idt[:],
                    rhs=prm[:, q * K1 * SLC:(q + 1) * K1 * SLC],
                    start=(q == 0), stop=(q == NSLC - 1))
            nc.vector.tensor_reduce(
                out=scores[:, t * K1:(t + 1) * K1],
                in_=acc[:].rearrange("p (k d) -> p k d", k=K1),
                axis=X, op=mybir.AluOpType.add)
        # this group's sigmoids (sigmoid table stays resident on ACT)
        w = ng * K1
        sl = slice(GSTART[g] * K1, GSTART[g] * K1 + w)
        s3 = scores[:, sl].rearrange("p (t k) -> p t k", k=K1)
        g3 = sig[:, sl].rearrange("p (t k) -> p t k", k=K1)
        nc.scalar.activation(out=g3[:, :, 0:1], in_=s3[:, :, 0:1],
                             func=AF.Sigmoid, scale=1.0)
        nc.scalar.activation(out=g3[:, :, 1:K1], in_=s3[:, :, 1:K1],
                             func=AF.Sigmoid, scale=-1.0)

    colsum = sc_pool.tile([P, 1], f32, tag="colsum")
    nc.scalar.activation(out=lg[:], in_=sig[:], func=AF.Ln, bias=eps_t[:],
                         accum_out=colsum[:])
    acc_ps = wm_pool.tile([1, 1], f32, tag="accf")
    nc.tensor.matmul(out=acc_ps[:], lhsT=colsum[:], rhs=ones[:],
                     start=True, stop=True)
    res = sc_pool.tile([1, 1], f32, tag="res")
    nc.vector.tensor_copy(out=res[:], in_=acc_ps[:])
    nc.sync.dma_start(out=out[:, :], in_=res[:])


def _build():
    nc = bacc.Bacc("TRN2", target_bir_lowering=False, debug=False)
    w_in = nc.dram_tensor("w_in", [V, D], DT_TAB, kind="ExternalInput")
    w_out = nc.dram_tensor("w_out", [V, D], DT_TAB, kind="ExternalInput")
    cidx = nc.dram_tensor("cidx", [N_C, P, C_GRP], mybir.dt.int32,
                          kind="ExternalInput")
    pnidx = nc.dram_tensor("pnidx", [P, NTILES * K1], mybir.dt.int32,
                           kind="ExternalInput")
    ident = nc.dram_tensor("ident", [P, P], DT_TAB, kind="ExternalInput")
    out = nc.dram_tensor("out", [1, 1], mybir.dt.float32,
                         kind="ExternalOutput")
    with tile.TileContext(nc) as tc:
        with ExitStack() as ctx:
            _body(ctx, tc, w_in, w_out, cidx, pnidx, ident, out)
    nc.compile()
    return nc


def _get_nc():
    global _NC
    if _NC is None:
        _NC = _build()
    return _NC


def _make_in_maps(inputs):
    center = np.asarray(inputs["center"]).astype(np.int32)
    pos = np.asarray(inputs["pos"]).astype(np.int32)
    neg = np.asarray(inputs["neg"]).astype(np.int32)
    wi = np.asarray(inputs["W_in"]).astype(NP_TAB)
    wo = np.asarray(inputs["W_out"]).astype(NP_TAB)
    ident = np.eye(P, dtype=NP_TAB)

    in_maps = []
    for c in range(NCORES):
        sl = slice(c * BPC, (c + 1) * BPC)
        # ce[g, p, j] = center index of batch tile t=g*C_GRP+j, partition p
        ce = center[sl].reshape(N_C, C_GRP, P).transpose(0, 2, 1)
        ce = np.ascontiguousarray(ce)
        # pn[p, t*K1+k] = pos/neg index of batch tile t, partition p
        pn = np.empty((NTILES, P, K1), np.int32)
        pn[:, :, 0] = pos[sl].reshape(NTILES, P)
        pn[:, :, 1:] = neg[sl].reshape(NTILES, P, NEG)
        pn = np.ascontiguousarray(
            pn.transpose(1, 0, 2).reshape(P, NTILES * K1))
        in_maps.append({"w_in": wi, "w_out": wo, "cidx": ce, "pnidx": pn,
                        "ident": ident})
    return in_maps


def kernel(center, pos, neg, W_in, W_out):
    global LAST_RESULTS
    in_maps = _make_in_maps(dict(center=center, pos=pos, neg=neg,
                                 W_in=W_in, W_out=W_out))
    nc = _get_nc()
    br = run_bass_kernel_spmd(nc, in_maps, core_ids=list(range(NCORES)),
                              trace=TRACE)
    LAST_RESULTS = br
    total = sum(float(r["out"][0, 0]) for r in br.results)
    return np.float32(-total / B)

